# revision 41
# baseline (speedup 1.0000x reference)
"""GQA decode attention (B=32, S=1, 32 Q heads / 8 KV heads, HD=128, T=4096)
for 8 Trainium2 NeuronCores, tensor-parallel over heads.

Per core g: 4 query heads (4g..4g+3) + KV head g.

v7 flash-decode schedule:
  - weights consolidated into 3 pre-arranged dram tensors (few big 8KB/line
    DMAs); K-cache prefetched right behind them; all DMA issues spread over
    sync/scalar/gpsimd queues
  - scores run c-major (chunk-major) so each PSUM bank finishes early; a
    LOCAL softmax (max m_c, exp, row-sum l_c) per bank hides under the next
    bank's matmuls
  - T is split in two halves. After half-0's scores, its chunks are rescaled
    to the half max (gamma_c = e^{m_c-m_H0}), transposed, and PV-half-0 runs
    INTERLEAVED with half-1's scores matmuls while V-half-0 streams next to
    K-half-1 -- the PE's PV work overlaps the K stream instead of bunching
    up at the end
  - the two PV accumulators merge with rank-1-broadcast alpha_H = e^{m_H-m}
    column scales; 1/rowsum is folded into the same final scale (p~ stays
    unnormalized end to end)
  - new-token k/v never touch the streamed caches: the score column is a
    DVE reduce scattered into scores[:,4095], the value column a rank-1
    correction on the merged accumulator
  - V cache in fp8 e3m4 (halves V DMA; ~1.2e-2 rel err, gate is 2e-2), all
    other operands fp16, accumulation fp32
  - wo preloaded during the V-half-1 stream; 8x4 chained matmuls +
    pipelined output DMA

Host pre-arranges K as [TC, 128, B*512] (c-major) and V as quad-batch
half tiles [8, 2, 128, 4*16*HD]. Partial outputs summed on host.
"""

import numpy as np
import ml_dtypes

B, DIM, NH, NKV, HD = 32, 4096, 32, 8, 128
T = 4096
NCORES = 8
HPC = NH // NCORES            # 4 query heads per core
OUTW = HPC * HD               # 512
ALPHA = float(1.0 / np.sqrt(HD))
DC = DIM // 128               # 32 contraction chunks for projections
TC = T // 512                 # 8 score chunks (512 wide)
PC = T // 128                 # 32 PV chunks (128 deep)
CH = TC // 2                  # 4 score chunks per half
PCH = PC // 2                 # 16 PV chunks per half

KBUFS = 5                     # K-cache tile depth ([128,8,512] fp16, 1MB)
VBUFS = 4                     # V quad-half tile depth (1MB each, fp8)
WARMN = 16                    # PE warm-up matmuls (p-state ramp)
V_FP8 = True                  # V cache in fp8 e3m4


def build_nc():
    import concourse.mybir as mybir
    import concourse.tile as tile
    from concourse import bacc

    f32 = mybir.dt.float32
    f16 = mybir.dt.float16
    vdt = mybir.dt.float8e3 if V_FP8 else f16
    X = mybir.AxisListType.X
    EXP = mybir.ActivationFunctionType.Exp
    SUB = mybir.AluOpType.subtract
    MAX = mybir.AluOpType.max

    nc = bacc.Bacc("TRN2", target_bir_lowering=False, debug=False,
                   num_devices=NCORES)

    xT = nc.dram_tensor("xT", [128, DC * B], f16, kind="ExternalInput")
    wq = nc.dram_tensor("wq", [128, DC * OUTW], f16, kind="ExternalInput")
    wkv = nc.dram_tensor("wkv", [128, DC * 2 * HD], f16, kind="ExternalInput")
    wo = nc.dram_tensor("wo", [128, HPC * DIM], f16, kind="ExternalInput")
    kt = nc.dram_tensor("kt", [TC, 128, B * 512], f16, kind="ExternalInput")
    vc = nc.dram_tensor("vc", [B // 4, 2, 128, 4 * PCH * HD], vdt,
                        kind="ExternalInput")
    csq = nc.dram_tensor("csq", [2, OUTW // 2], f32, kind="ExternalInput")
    csk = nc.dram_tensor("csk", [2, HD // 2], f32, kind="ExternalInput")
    ones16 = nc.dram_tensor("ones16", [1, 128], f16, kind="ExternalInput")
    ones32 = nc.dram_tensor("ones32", [1, 128], f32, kind="ExternalInput")
    iden = nc.dram_tensor("iden", [128, 128], f32, kind="ExternalInput")
    iden16 = nc.dram_tensor("iden16", [128, 128], f16, kind="ExternalInput")
    outp = nc.dram_tensor("outp", [B, DIM], f32, kind="ExternalOutput")

    with tile.TileContext(nc) as tc:
        with (
            tc.tile_pool(name="pp", bufs=1) as pp,
            tc.tile_pool(name="vqp", bufs=VBUFS) as vqp,
            tc.tile_pool(name="mp", bufs=2) as mp,
            tc.tile_pool(name="outp_pool", bufs=2) as outpp,
        ):
            # PSUM pools for the PV accumulator and p~ transposes are opened
            # before ktp so the pool stack stays LIFO through ktp's close
            psP_cm = tc.tile_pool(name="psP", bufs=1, space="PSUM")
            psP = psP_cm.__enter__()
            psT_cm = tc.tile_pool(name="psT", bufs=2, space="PSUM")
            psT = psT_cm.__enter__()
            ktp_cm = tc.tile_pool(name="ktp", bufs=KBUFS)
            ktp = ktp_cm.__enter__()

            # ------- constants (scalar queue)
            xT_sb = pp.tile([128, DC, B], f16, tag="xT_sb")
            nc.scalar.dma_start(xT_sb,
                                xT[:].rearrange("p (dc b) -> p dc b", b=B))
            iden_sb = pp.tile([128, 128], f32, tag="iden_sb")
            nc.scalar.dma_start(iden_sb, iden[:])
            iden16_sb = pp.tile([128, 128], f16, tag="iden16_sb")
            nc.scalar.dma_start(iden16_sb, iden16[:])
            ones16_sb = pp.tile([1, 128], f16, tag="ones16_sb")
            nc.scalar.dma_start(ones16_sb, ones16[:])
            ones32_sb = pp.tile([1, 128], f32, tag="ones32_sb")
            nc.scalar.dma_start(ones32_sb, ones32[:])
            cq32 = pp.tile([B, OUTW // 2], f32, tag="cq32")
            nc.scalar.dma_start(cq32,
                                csq[0:1, :].to_broadcast([B, OUTW // 2]))
            sq32 = pp.tile([B, OUTW // 2], f32, tag="sq32")
            nc.scalar.dma_start(sq32,
                                csq[1:2, :].to_broadcast([B, OUTW // 2]))
            ck32 = pp.tile([B, HD // 2], f32, tag="ck32")
            nc.scalar.dma_start(ck32, csk[0:1, :].to_broadcast([B, HD // 2]))
            sk32 = pp.tile([B, HD // 2], f32, tag="sk32")
            nc.scalar.dma_start(sk32, csk[1:2, :].to_broadcast([B, HD // 2]))
            zero1 = pp.tile([128, 1], f32, tag="zero1")
            nc.vector.memset(zero1, 0.0)
            zero16 = pp.tile([128, 1], f16, tag="zero16")
            nc.vector.memset(zero16, 0.0)

            # PE warm-up: dummy matmuls (no DMA deps) ramp the tensor
            # engine's p-state while the weight DMAs are in flight
            warm = pp.tile([128, 512], f16, tag="warm")
            nc.vector.memset(warm, 0.5)

            qxall = pp.tile([128, B * 128], f16, tag="qxall")
            nc.vector.tensor_copy(
                qxall, zero1[:, 0:1].to_broadcast([128, B * 128]))

            kt_tiles = {}
            snew = pp.tile([B, HPC], f32, tag="snew")
            snew_col = pp.tile([128, 1], f32, tag="snew_col")
            qrot = pp.tile([B, OUTW], f32, tag="qrot")
            krot = pp.tile([B, HD], f32, tag="krot")
            vnewT_sb = pp.tile([128, B], f32, tag="vnewT_sb")
            qT_sb = pp.tile([128, HPC, B], f32, tag="qT_sb")

            ktv = kt[:].rearrange("c p (bg j n) -> c p bg j n", n=512, j=8)
            vcv = vc[:].rearrange("q h p (a c d) -> q h p a c d",
                                  d=HD, c=PCH)

            # ------- phase A: weights in a scoped pool (freed afterwards)
            with tc.tile_pool(name="wpool", bufs=1) as wpool:
                wq_sb = wpool.tile([128, DC, OUTW], f16, tag="wq_sb")
                wqv = wq[:].rearrange("p (dc o) -> p dc o", o=OUTW)
                for i in range(4):
                    nc.gpsimd.dma_start(wq_sb[:, 8 * i:8 * (i + 1), :],
                                        wqv[:, 8 * i:8 * (i + 1), :])
                wkv_sb = wpool.tile([128, DC, 2 * HD], f16, tag="wkv_sb")
                wkvv = wkv[:].rearrange("p (dc o) -> p dc o", o=2 * HD)
                for i in range(2):
                    nc.gpsimd.dma_start(wkv_sb[:, 16 * i:16 * (i + 1), :],
                                        wkvv[:, 16 * i:16 * (i + 1), :])

                # K-cache prefetch: c-major (chunk, batch-group-of-8) tiles
                for t in range(KBUFS):
                    tkb = ktp.tile([128, 8, 512], f16, tag="ktb",
                                   name=f"ktb{t}")
                    c, bg = divmod(t, 4)
                    nc.sync.dma_start(tkb, ktv[c, :, bg])
                    kt_tiles[t] = tkb

                with tc.tile_pool(name="psW", bufs=1, space="PSUM") as psW:
                    psw = psW.tile([128, 512], f32, tag="psw")
                    for i in range(WARMN):
                        nc.tensor.matmul(psw, warm[:, 0:128], warm,
                                         start=True, stop=True)

                with tc.tile_pool(name="psA", bufs=1, space="PSUM") as psA:
                    psq = psA.tile([B, OUTW], f32, tag="psq")
                    for dc in range(DC):
                        nc.tensor.matmul(psq, xT_sb[:, dc, :],
                                         wq_sb[:, dc, :],
                                         start=(dc == 0), stop=(dc == DC - 1))
                    pskv = psA.tile([B, 2 * HD], f32, tag="pskv")
                    for dc in range(DC):
                        nc.tensor.matmul(pskv, xT_sb[:, dc, :],
                                         wkv_sb[:, dc, :],
                                         start=(dc == 0), stop=(dc == DC - 1))

                    q_sb = pp.tile([B, OUTW], f32, tag="q_sb")
                    nc.vector.tensor_copy(q_sb, psq)
                    k_sb = pp.tile([B, HD], f32, tag="k_sb")
                    nc.vector.tensor_copy(k_sb, pskv[:, 0:HD])
                    vnew_sb = pp.tile([B, HD], f32, tag="vnew_sb")
                    nc.vector.tensor_copy(vnew_sb, pskv[:, HD:2 * HD])

                    # rope on q (scaled by alpha via csq) and k (unscaled)
                    tA = mp.tile([B, OUTW // 2], f32, tag="ropetmp", name="tA")
                    tB = mp.tile([B, OUTW // 2], f32, tag="ropetmp", name="tB")
                    qe, qo = q_sb[:, 0::2], q_sb[:, 1::2]
                    nc.vector.tensor_mul(tA, qe, cq32)
                    nc.vector.tensor_mul(tB, qo, sq32)
                    nc.vector.tensor_tensor(qrot[:, 0::2], tA, tB, SUB)
                    tC = mp.tile([B, OUTW // 2], f32, tag="ropetmp", name="tC")
                    tD = mp.tile([B, OUTW // 2], f32, tag="ropetmp", name="tD")
                    nc.vector.tensor_mul(tC, qe, sq32)
                    nc.vector.tensor_mul(tD, qo, cq32)
                    nc.vector.tensor_add(qrot[:, 1::2], tC, tD)

                    uA = mp.tile([B, HD // 2], f32, tag="kropetmp", name="uA")
                    uB = mp.tile([B, HD // 2], f32, tag="kropetmp", name="uB")
                    ke, ko = k_sb[:, 0::2], k_sb[:, 1::2]
                    nc.vector.tensor_mul(uA, ke, ck32)
                    nc.vector.tensor_mul(uB, ko, sk32)
                    nc.vector.tensor_tensor(krot[:, 0::2], uA, uB, SUB)
                    uC = mp.tile([B, HD // 2], f32, tag="kropetmp", name="uC")
                    uD = mp.tile([B, HD // 2], f32, tag="kropetmp", name="uD")
                    nc.vector.tensor_mul(uC, ke, sk32)
                    nc.vector.tensor_mul(uD, ko, ck32)
                    nc.vector.tensor_add(krot[:, 1::2], uC, uD)

                    # new-token scores: snew[b,h] = sum_d qrot[b,h,d]*krot[b,d]
                    tmp4 = mp.tile([B, HPC, HD], f32, tag="tmp4")
                    nc.vector.tensor_mul(
                        tmp4,
                        qrot[:].rearrange("b (h d) -> b h d", d=HD),
                        krot[:, None, :].to_broadcast([B, HPC, HD]))
                    for h in range(HPC):
                        nc.vector.reduce_sum(snew[:, h:h + 1], tmp4[:, h, :],
                                             axis=X)
                    nc.sync.dma_start(snew_col, snew[:])

                    # transpose q per head -> qxall zero-padded blocks
                    for h in range(HPC):
                        pst = psA.tile([128, B], f32, tag="pstA",
                                       name=f"pstA{h}")
                        nc.tensor.transpose(pst, qrot[:, h * HD:(h + 1) * HD],
                                            iden_sb[0:B, 0:B])
                        nc.vector.tensor_copy(qT_sb[:, h, :], pst)
                    pstv = psA.tile([128, B], f32, tag="pstA")
                    nc.tensor.transpose(pstv, vnew_sb, iden_sb[0:B, 0:B])
                    nc.vector.tensor_copy(vnewT_sb, pstv)

                    for b in range(B):
                        nc.vector.tensor_copy(
                            qxall[:, 128 * b + HPC * b:128 * b
                                  + HPC * (b + 1)],
                            qT_sb[:, :, b])

            # ------- scores + local softmax + interleaved PV (flash halves)
            p16 = pp.tile([128, T], f16, tag="p16")
            maxv = pp.tile([128, 1], f32, tag="maxv")
            negmax = pp.tile([128, 1], f32, tag="negmax")
            sums = pp.tile([128, 1], f32, tag="sums")
            recip = pp.tile([128, 1], f32, tag="recip")
            prow16 = pp.tile([1, 128], f16, tag="prow16")
            rT32 = pp.tile([1, 128], f32, tag="rT32")
            pT = pp.tile([128, PC, 128], f16, tag="pT")
            mh = [pp.tile([128, 1], f32, tag=f"mh{H}", name=f"mh{H}")
                  for H in range(2)]
            v_tiles = {}
            m_c, l_c = [], []

            psat = psP.tile([128, 2, B * HPC], f32, tag="psat")
            psB_cm = tc.tile_pool(name="psB", bufs=1, space="PSUM")
            psB = psB_cm.__enter__()

            def score_chunk(c, pqk):
                """32 scores matmuls for chunk c + local softmax."""
                for bg in range(4):
                    t_i = c * 4 + bg
                    tkb = kt_tiles.pop(t_i)
                    for j in range(8):
                        b = bg * 8 + j
                        nc.tensor.matmul(
                            pqk[:, c % CH, :],
                            qxall[:, 128 * b:128 * (b + 1)],
                            tkb[:, j, :],
                            start=(b == 0), stop=(b == B - 1),
                            skip_group_check=True)
                    nt = t_i + KBUFS
                    if nt < 4 * TC:
                        nc2, nbg = divmod(nt, 4)
                        t2 = ktp.tile([128, 8, 512], f16, tag="ktb",
                                      name=f"ktb{nt}")
                        nc.sync.dma_start(t2, ktv[nc2, :, nbg])
                        kt_tiles[nt] = t2
                if c == TC - 1:
                    # zero the stale col-4095 score: its exp contributes
                    # only e^-m_c to the row sum (negligible)
                    nc.vector.tensor_copy(pqk[:, CH - 1, 511:512], zero1)
                mc = mp.tile([128, 1], f32, tag="mxc", name=f"mx{c}",
                             bufs=TC)
                nc.vector.reduce_max(mc, pqk[:, c % CH, :], axis=X)
                ngc = mp.tile([128, 1], f32, tag="ngc", name=f"ng{c}",
                              bufs=TC)
                nc.vector.tensor_scalar_mul(ngc, mc, -1.0)
                lc = mp.tile([128, 1], f32, tag="sumc", name=f"sum{c}",
                             bufs=TC)
                nc.scalar.activation(p16[:, c * 512:(c + 1) * 512],
                                     pqk[:, c % CH, :], EXP, bias=ngc,
                                     scale=1.0, accum_out=lc)
                m_c.append(mc)
                l_c.append(lc)

            def rescale_transpose_half(H):
                """gamma_c rescale to the half max + pT transposes."""
                nc.vector.tensor_tensor(mh[H], m_c[4 * H], m_c[4 * H + 1],
                                        MAX)
                nc.vector.tensor_tensor(mh[H], mh[H], m_c[4 * H + 2], MAX)
                nc.vector.tensor_tensor(mh[H], mh[H], m_c[4 * H + 3], MAX)
                for c in range(4 * H, 4 * H + 4):
                    gs = mp.tile([128, 1], f32, tag="gsc", name=f"gs{c}",
                                 bufs=TC)
                    nc.vector.tensor_tensor(gs, m_c[c], mh[H], SUB)
                    gc = mp.tile([128, 1], f32, tag="gc", name=f"g{c}",
                                 bufs=TC)
                    nc.scalar.activation(gc, gs, EXP)
                    g_c.append(gc)
                    nc.vector.tensor_scalar_mul(
                        p16[:, c * 512:(c + 1) * 512],
                        p16[:, c * 512:(c + 1) * 512], gc)
                for c2 in range(PCH * H, PCH * (H + 1)):
                    pstx = psT.tile([128, 128], f16, tag="pstx",
                                    name=f"pstx{c2}")
                    nc.tensor.transpose(pstx, p16[:, c2 * 128:(c2 + 1) * 128],
                                        iden16_sb)
                    nc.vector.tensor_copy(pT[:, c2, :], pstx)

            def pv_batches(H, b0, b1):
                """PV matmuls for batches [b0,b1) of half H + V recycling.

                V tiles stream in consumption order: flat index 8*H + bq.
                """
                for b in range(b0, b1):
                    bq, lane = b // 4, b % 4
                    t_i = 8 * H + bq
                    vb = v_tiles[t_i]
                    for cl in range(PCH):
                        nc.tensor.matmul(
                            psat[:, H, HPC * b:HPC * (b + 1)],
                            vb[:, lane, cl, :],
                            pT[:, PCH * H + cl, HPC * b:HPC * (b + 1)],
                            start=(cl == 0), stop=(cl == PCH - 1),
                            skip_group_check=True)
                    if lane == 3:
                        del v_tiles[t_i]
                        nt = t_i + VBUFS
                        if nt < 16:
                            nH, nbq = divmod(nt, 8)
                            v2t = vqp.tile([128, 4, PCH, HD], vdt, tag="vq",
                                           name=f"vq{nt}")
                            nc.gpsimd.dma_start(v2t, vcv[nbq, nH])
                            v_tiles[nt] = v2t

            g_c = []
            # half 0 scores
            pqk0 = psB.tile([128, CH, 512], f32, tag="pqk", name="pqk0")
            for c in range(CH):
                score_chunk(c, pqk0)
            # V tile order: flat index 8*H+bq (all half-0 quads first);
            # prefetch first VBUFS (gpsimd queue) -- streams beside K-half-1
            for t in range(VBUFS):
                vb = vqp.tile([128, 4, PCH, HD], vdt, tag="vq",
                              name=f"vq{t}")
                nc.gpsimd.dma_start(vb, vcv[t, 0])
                v_tiles[t] = vb
            rescale_transpose_half(0)

            # half 1 scores interleaved with PV half 0
            pqk1 = psB.tile([128, CH, 512], f32, tag="pqk", name="pqk1")
            for c in range(CH, TC):
                score_chunk(c, pqk1)
                pv_batches(0, (c - CH) * 8, (c - CH) * 8 + 8)
            psB_cm.__exit__(None, None, None)
            ktp_cm.__exit__(None, None, None)

            # wo preload in the freed K region
            wopool_cm = tc.tile_pool(name="wopool", bufs=1)
            wopool = wopool_cm.__enter__()
            wo_sb = wopool.tile([128, HPC, DIM], f16, tag="wo_sb")
            wov = wo[:].rearrange("p (h o) -> p h o", o=DIM)
            for h in range(HPC):
                nc.gpsimd.dma_start(wo_sb[:, h, :], wov[:, h, :])

            rescale_transpose_half(1)
            # PV half 1
            pv_batches(1, 0, B)

            # ------- merge halves + correction + output projection
            nc.vector.tensor_tensor(maxv, mh[0], mh[1], MAX)
            nc.vector.tensor_tensor(maxv, maxv, snew_col, MAX)
            nc.vector.tensor_scalar_mul(negmax, maxv, -1.0)
            # alpha_H = e^{m_H - m}; row sums = sum_H alpha_H sum_c l_c g_c
            aH = []
            for H in range(2):
                as_ = mp.tile([128, 1], f32, tag="asH", name=f"as{H}",
                              bufs=2)
                nc.vector.tensor_tensor(as_, mh[H], maxv, SUB)
                a_ = mp.tile([128, 1], f32, tag="aH", name=f"a{H}", bufs=2)
                nc.scalar.activation(a_, as_, EXP)
                aH.append(a_)
            lg = mp.tile([128, 1], f32, tag="lg")
            lh = mp.tile([128, 1], f32, tag="lh")
            first = True
            for H in range(2):
                for c in range(4 * H, 4 * H + 4):
                    nc.vector.tensor_mul(lg, l_c[c], g_c[c])
                    if c % 4 == 0:
                        nc.vector.tensor_copy(lh, lg)
                    else:
                        nc.vector.tensor_add(lh, lh, lg)
                nc.vector.tensor_mul(lh, lh, aH[H])
                if first:
                    nc.vector.tensor_copy(sums, lh)
                    first = False
                else:
                    nc.vector.tensor_add(sums, sums, lh)
            # new-token exp -> p~ col (global max)
            nc.scalar.activation(p16[:, T - 1:T], snew_col, EXP, bias=negmax,
                                 scale=1.0)
            pcol32 = mp.tile([128, 1], f32, tag="pcol32")
            nc.vector.tensor_copy(pcol32, p16[:, T - 1:T])
            nc.vector.tensor_add(sums, sums, pcol32)
            nc.vector.reciprocal(recip, sums)

            with tc.tile_pool(name="psC", bufs=2, space="PSUM") as psC:
                psr = psC.tile([1, 128], f16, tag="psrow", bufs=1,
                               name="psr")
                nc.tensor.transpose(psr, p16[:, T - 1:T], iden16_sb)
                nc.vector.tensor_copy(prow16, psr)

                pstr = psC.tile([1, 128], f32, tag="psrow", bufs=1,
                                name="pstr")
                nc.tensor.transpose(pstr, recip, iden_sb)
                nc.vector.tensor_copy(rT32, pstr)

                # rank-1 column broadcasts: alpha_0, alpha_1, p~row, recip
                a0T = mp.tile([1, 128], f32, tag="a0T")
                psa0 = psC.tile([1, 128], f32, tag="psrow", bufs=1,
                                name="psa0")
                nc.tensor.transpose(psa0, aH[0], iden_sb)
                nc.vector.tensor_copy(a0T, psa0)
                a1T = mp.tile([1, 128], f32, tag="a1T")
                psa1 = psC.tile([1, 128], f32, tag="psrow", bufs=1,
                                name="psa1")
                nc.tensor.transpose(psa1, aH[1], iden_sb)
                nc.vector.tensor_copy(a1T, psa1)

                # serialized rank-1 broadcasts (one PSUM bank each, consumer
                # immediately after producer so the bank can rotate)
                psbc1 = psC.tile([128, 128], f32, tag="psbc", bufs=1,
                                 name="psbc1")
                nc.tensor.matmul(psbc1, ones16_sb, prow16)
                corrT = mp.tile([128, B, HPC], f32, tag="corrT")
                nc.vector.tensor_mul(
                    corrT,
                    vnewT_sb[:, :, None].to_broadcast([128, B, HPC]),
                    psbc1[:].rearrange("d (b h) -> d b h", h=HPC))
                psba0 = psC.tile([128, 128], f32, tag="psba", bufs=1,
                                 name="psba0")
                nc.tensor.matmul(psba0, ones32_sb, a0T)
                at_f = mp.tile([128, B * HPC], f32, tag="at_f")
                nc.vector.tensor_copy(at_f, psat[:, 0, :])
                nc.vector.tensor_mul(at_f, at_f, psba0)
                psba1 = psC.tile([128, 128], f32, tag="psba", bufs=1,
                                 name="psba1")
                nc.tensor.matmul(psba1, ones32_sb, a1T)
                at_g = mp.tile([128, B * HPC], f32, tag="at_g")
                nc.vector.tensor_copy(at_g, psat[:, 1, :])
                nc.vector.tensor_mul(at_g, at_g, psba1)
                nc.vector.tensor_add(at_f, at_f, at_g)
                nc.vector.tensor_add(
                    at_f, at_f, corrT[:].rearrange("d b h -> d (b h)"))
                psbc2 = psC.tile([128, 128], f32, tag="psbc", bufs=1,
                                 name="psbc2")
                nc.tensor.matmul(psbc2, ones32_sb, rT32)
                attnT = pp.tile([128, B * HPC], f16, tag="attnT")
                nc.vector.tensor_mul(attnT, at_f, psbc2)

                # out projection
                for ncc in range(8):
                    pso = psC.tile([B, 512], f32, tag="pso", name=f"pso{ncc}")
                    for h in range(HPC):
                        nc.tensor.matmul(
                            pso, attnT[:, h::HPC],
                            wo_sb[:, h, ncc * 512:(ncc + 1) * 512],
                            start=(h == 0), stop=(h == HPC - 1))
                    osb = outpp.tile([B, 512], f32, tag="osb",
                                     name=f"osb{ncc}")
                    nc.vector.tensor_copy(osb, pso)
                    nc.sync.dma_start(outp[:, ncc * 512:(ncc + 1) * 512], osb)

            wopool_cm.__exit__(None, None, None)
            psT_cm.__exit__(None, None, None)
            psP_cm.__exit__(None, None, None)

    nc.compile()
    return nc


def make_in_maps(inputs):
    x = np.asarray(inputs["x"], np.float32).reshape(B, DIM)
    cache_k = np.asarray(inputs["cache_k"], np.float32)
    cache_v = np.asarray(inputs["cache_v"], np.float32)
    wq = np.asarray(inputs["wq"], np.float32)
    wk = np.asarray(inputs["wk"], np.float32)
    wv = np.asarray(inputs["wv"], np.float32)
    wo = np.asarray(inputs["wo"], np.float32)
    cos = np.asarray(inputs["freqs_cos"], np.float32).reshape(-1)
    sin = np.asarray(inputs["freqs_sin"], np.float32).reshape(-1)

    f16 = np.float16
    vdt = ml_dtypes.float8_e3m4 if V_FP8 else f16
    xT = np.ascontiguousarray(
        x.T.reshape(DC, 128, B).transpose(1, 0, 2)
        .reshape(128, DC * B)).astype(f16)                     # [128, DC*B]
    csq = np.ascontiguousarray(
        np.stack([np.tile(cos, HPC), np.tile(sin, HPC)]) * ALPHA)
    csk = np.ascontiguousarray(np.stack([cos, sin]))
    ones16v = np.ones((1, 128), f16)
    ones32v = np.ones((1, 128), np.float32)
    idenv = np.eye(128, dtype=np.float32)
    iden16v = np.eye(128, dtype=f16)

    v8 = cache_v.astype(vdt)                                   # quantize once

    in_maps = []
    for g in range(NCORES):
        wq_g = wq[:, g * OUTW:(g + 1) * OUTW]
        wq_pre = np.ascontiguousarray(
            wq_g.reshape(DC, 128, OUTW).transpose(1, 0, 2)
            .reshape(128, DC * OUTW)).astype(f16)
        wk_r = wk[:, g * HD:(g + 1) * HD].reshape(DC, 128, HD)
        wv_r = wv[:, g * HD:(g + 1) * HD].reshape(DC, 128, HD)
        wkv_pre = np.ascontiguousarray(
            np.stack([wk_r, wv_r], axis=2).transpose(1, 0, 2, 3)
            .reshape(128, DC * 2 * HD)).astype(f16)
        wo_g = wo[g * OUTW:(g + 1) * OUTW, :]
        wo_pre = np.ascontiguousarray(
            wo_g.reshape(HPC, 128, DIM).transpose(1, 0, 2)
            .reshape(128, HPC * DIM)).astype(f16)
        kt_g = np.ascontiguousarray(
            cache_k[:, :, g, :].reshape(B, TC, 512, HD)
            .transpose(1, 3, 0, 2)
            .reshape(TC, 128, B * 512)).astype(f16)    # [TC,128,B*512]
        # V: quad-batch half tiles [bq, H, r, lane, chunk, d]
        v_g = np.ascontiguousarray(
            v8[:, :, g, :].reshape(B // 4, 4, 2, PCH, 128, HD)
            .transpose(0, 2, 4, 1, 3, 5)
            .reshape(B // 4, 2, 128, 4 * PCH * HD))
        in_maps.append({
            "xT": xT,
            "wq": wq_pre,
            "wkv": wkv_pre,
            "wo": wo_pre,
            "kt": kt_g,
            "vc": v_g,
            "csq": csq,
            "csk": csk,
            "ones16": ones16v,
            "ones32": ones32v,
            "iden": idenv,
            "iden16": iden16v,
        })
    return in_maps


_NC_CACHE = []


def run(inputs, trace=False, **kwargs):
    from concourse.bass_utils import run_bass_kernel_spmd
    if not _NC_CACHE:
        _NC_CACHE.append(build_nc())
    nc = _NC_CACHE[0]
    in_maps = make_in_maps(inputs)
    res = run_bass_kernel_spmd(nc, in_maps, core_ids=list(range(NCORES)),
                               trace=trace, **kwargs)
    partials = np.stack([r["outp"] for r in res.results])      # [8, B, DIM]
    out = partials.sum(axis=0, dtype=np.float64).astype(np.float32)
    return out.reshape(B, 1, DIM), res


def kernel(**inputs):
    out, _ = run(inputs)
    return out


# revision 48
# speedup vs baseline: 1.0578x; 1.0578x over previous
"""GQA decode attention (B=32, S=1, 32 Q heads / 8 KV heads, HD=128, T=4096)
for 8 Trainium2 NeuronCores, tensor-parallel over heads.

Per core g: 4 query heads (4g..4g+3) + KV head g.

v7 flash-decode schedule:
  - weights consolidated into 3 pre-arranged dram tensors (few big 8KB/line
    DMAs); K-cache prefetched right behind them; all DMA issues spread over
    sync/scalar/gpsimd queues
  - scores run c-major (chunk-major) so each PSUM bank finishes early; a
    LOCAL softmax (max m_c, exp, row-sum l_c) per bank hides under the next
    bank's matmuls
  - T is split in two halves. After half-0's scores, its chunks are rescaled
    to the half max (gamma_c = e^{m_c-m_H0}), transposed, and PV-half-0 runs
    INTERLEAVED with half-1's scores matmuls while V-half-0 streams next to
    K-half-1 -- the PE's PV work overlaps the K stream instead of bunching
    up at the end
  - the two PV accumulators merge with rank-1-broadcast alpha_H = e^{m_H-m}
    column scales; 1/rowsum is folded into the same final scale (p~ stays
    unnormalized end to end)
  - new-token k/v never touch the streamed caches: the score column is a
    DVE reduce scattered into scores[:,4095], the value column a rank-1
    correction on the merged accumulator
  - V cache in fp8 e3m4 (halves V DMA; ~1.2e-2 rel err, gate is 2e-2), all
    other operands fp16, accumulation fp32
  - wo preloaded during the V-half-1 stream; 8x4 chained matmuls +
    pipelined output DMA

Host pre-arranges K as [TC, 128, B*512] (c-major) and V as quad-batch
half tiles [8, 2, 128, 4*16*HD]. Partial outputs summed on host.
"""

import numpy as np
import ml_dtypes

B, DIM, NH, NKV, HD = 32, 4096, 32, 8, 128
T = 4096
NCORES = 8
HPC = NH // NCORES            # 4 query heads per core
OUTW = HPC * HD               # 512
ALPHA = float(1.0 / np.sqrt(HD))
DC = DIM // 128               # 32 contraction chunks for projections
TC = T // 512                 # 8 score chunks (512 wide)
PC = T // 128                 # 32 PV chunks (128 deep)
CH = TC // 2                  # 4 score chunks per half
PCH = PC // 2                 # 16 PV chunks per half

KBUFS = 3                     # K-cache tile depth ([128,8,2,512] fp16, 2MB)
VBUFS = 4                     # V quad-half tile depth (1MB each, fp8)
WARMN = 16                    # PE warm-up matmuls (p-state ramp)
V_FP8 = True                  # V cache in fp8 e3m4


def build_nc():
    import concourse.mybir as mybir
    import concourse.tile as tile
    from concourse import bacc

    f32 = mybir.dt.float32
    f16 = mybir.dt.float16
    vdt = mybir.dt.float8e3 if V_FP8 else f16
    X = mybir.AxisListType.X
    EXP = mybir.ActivationFunctionType.Exp
    SUB = mybir.AluOpType.subtract
    MAX = mybir.AluOpType.max

    nc = bacc.Bacc("TRN2", target_bir_lowering=False, debug=False,
                   num_devices=NCORES)

    xT = nc.dram_tensor("xT", [128, DC * B], f16, kind="ExternalInput")
    wq = nc.dram_tensor("wq", [128, DC * OUTW], f16, kind="ExternalInput")
    wkv = nc.dram_tensor("wkv", [128, DC * 2 * HD], f16, kind="ExternalInput")
    wo = nc.dram_tensor("wo", [128, HPC * DIM], f16, kind="ExternalInput")
    kt = nc.dram_tensor("kt", [16, 128, 8 * 2 * 512], f16,
                        kind="ExternalInput")
    vc = nc.dram_tensor("vc", [B // 4, 2, 128, 4 * PCH * HD], vdt,
                        kind="ExternalInput")
    csq = nc.dram_tensor("csq", [2, OUTW // 2], f32, kind="ExternalInput")
    csk = nc.dram_tensor("csk", [2, HD // 2], f32, kind="ExternalInput")
    ones16 = nc.dram_tensor("ones16", [1, 128], f16, kind="ExternalInput")
    ones32 = nc.dram_tensor("ones32", [1, 128], f32, kind="ExternalInput")
    iden = nc.dram_tensor("iden", [128, 128], f32, kind="ExternalInput")
    iden16 = nc.dram_tensor("iden16", [128, 128], f16, kind="ExternalInput")
    outp = nc.dram_tensor("outp", [B, DIM], f32, kind="ExternalOutput")

    with tile.TileContext(nc) as tc:
        with (
            tc.tile_pool(name="pp", bufs=1) as pp,
            tc.tile_pool(name="vqp", bufs=VBUFS) as vqp,
            tc.tile_pool(name="mp", bufs=2) as mp,
            tc.tile_pool(name="outp_pool", bufs=2) as outpp,
        ):
            # PSUM pools for the PV accumulator and p~ transposes are opened
            # before ktp so the pool stack stays LIFO through ktp's close
            psP_cm = tc.tile_pool(name="psP", bufs=1, space="PSUM")
            psP = psP_cm.__enter__()
            psT_cm = tc.tile_pool(name="psT", bufs=2, space="PSUM")
            psT = psT_cm.__enter__()
            ktp_cm = tc.tile_pool(name="ktp", bufs=KBUFS)
            ktp = ktp_cm.__enter__()

            # ------- constants (scalar queue)
            xT_sb = pp.tile([128, DC, B], f16, tag="xT_sb")
            nc.scalar.dma_start(xT_sb,
                                xT[:].rearrange("p (dc b) -> p dc b", b=B))
            iden_sb = pp.tile([128, 128], f32, tag="iden_sb")
            nc.scalar.dma_start(iden_sb, iden[:])
            iden16_sb = pp.tile([128, 128], f16, tag="iden16_sb")
            nc.scalar.dma_start(iden16_sb, iden16[:])
            ones16_sb = pp.tile([1, 128], f16, tag="ones16_sb")
            nc.scalar.dma_start(ones16_sb, ones16[:])
            ones32_sb = pp.tile([1, 128], f32, tag="ones32_sb")
            nc.scalar.dma_start(ones32_sb, ones32[:])
            cq32 = pp.tile([B, OUTW // 2], f32, tag="cq32")
            nc.scalar.dma_start(cq32,
                                csq[0:1, :].to_broadcast([B, OUTW // 2]))
            sq32 = pp.tile([B, OUTW // 2], f32, tag="sq32")
            nc.scalar.dma_start(sq32,
                                csq[1:2, :].to_broadcast([B, OUTW // 2]))
            ck32 = pp.tile([B, HD // 2], f32, tag="ck32")
            nc.scalar.dma_start(ck32, csk[0:1, :].to_broadcast([B, HD // 2]))
            sk32 = pp.tile([B, HD // 2], f32, tag="sk32")
            nc.scalar.dma_start(sk32, csk[1:2, :].to_broadcast([B, HD // 2]))
            zero1 = pp.tile([128, 1], f32, tag="zero1")
            nc.vector.memset(zero1, 0.0)
            zero16 = pp.tile([128, 1], f16, tag="zero16")
            nc.vector.memset(zero16, 0.0)

            # PE warm-up: dummy matmuls (no DMA deps) ramp the tensor
            # engine's p-state while the weight DMAs are in flight
            warm = pp.tile([128, 512], f16, tag="warm")
            nc.vector.memset(warm, 0.5)

            qxall = pp.tile([128, B * 128], f16, tag="qxall")
            nc.vector.tensor_copy(
                qxall, zero1[:, 0:1].to_broadcast([128, B * 128]))

            kt_tiles = {}
            snew = pp.tile([B, HPC], f32, tag="snew")
            snew_col = pp.tile([128, 1], f32, tag="snew_col")
            qrot = pp.tile([B, OUTW], f32, tag="qrot")
            krot = pp.tile([B, HD], f32, tag="krot")
            vnewT_sb = pp.tile([128, B], f32, tag="vnewT_sb")
            qT_sb = pp.tile([128, HPC, B], f32, tag="qT_sb")

            # K tile (H, pass, bg): [128, 8 batches, 2 chunks, 512]
            ktv = kt[:].rearrange("t p (j c n) -> t p j c n", n=512, c=2)
            vcv = vc[:].rearrange("q h p (a c d) -> q h p a c d",
                                  d=HD, c=PCH)

            # ------- phase A: weights in a scoped pool (freed afterwards)
            with tc.tile_pool(name="wpool", bufs=1) as wpool:
                wq_sb = wpool.tile([128, DC, OUTW], f16, tag="wq_sb")
                wqv = wq[:].rearrange("p (dc o) -> p dc o", o=OUTW)
                for i in range(4):
                    nc.gpsimd.dma_start(wq_sb[:, 8 * i:8 * (i + 1), :],
                                        wqv[:, 8 * i:8 * (i + 1), :])
                wkv_sb = wpool.tile([128, DC, 2 * HD], f16, tag="wkv_sb")
                wkvv = wkv[:].rearrange("p (dc o) -> p dc o", o=2 * HD)
                for i in range(2):
                    nc.gpsimd.dma_start(wkv_sb[:, 16 * i:16 * (i + 1), :],
                                        wkvv[:, 16 * i:16 * (i + 1), :])

                # K-cache prefetch: (half, pass, batch-group) tiles
                for t in range(KBUFS):
                    tkb = ktp.tile([128, 8, 2, 512], f16, tag="ktb",
                                   name=f"ktb{t}")
                    nc.sync.dma_start(tkb, ktv[t])
                    kt_tiles[t] = tkb

                with tc.tile_pool(name="psW", bufs=1, space="PSUM") as psW:
                    psw = psW.tile([128, 512], f32, tag="psw")
                    for i in range(WARMN):
                        nc.tensor.matmul(psw, warm[:, 0:128], warm,
                                         start=True, stop=True)

                with tc.tile_pool(name="psA", bufs=1, space="PSUM") as psA:
                    psq = psA.tile([B, OUTW], f32, tag="psq")
                    for dc in range(DC):
                        nc.tensor.matmul(psq, xT_sb[:, dc, :],
                                         wq_sb[:, dc, :],
                                         start=(dc == 0), stop=(dc == DC - 1))
                    pskv = psA.tile([B, 2 * HD], f32, tag="pskv")
                    for dc in range(DC):
                        nc.tensor.matmul(pskv, xT_sb[:, dc, :],
                                         wkv_sb[:, dc, :],
                                         start=(dc == 0), stop=(dc == DC - 1))

                    q_sb = pp.tile([B, OUTW], f32, tag="q_sb")
                    nc.vector.tensor_copy(q_sb, psq)
                    k_sb = pp.tile([B, HD], f32, tag="k_sb")
                    nc.vector.tensor_copy(k_sb, pskv[:, 0:HD])
                    vnew_sb = pp.tile([B, HD], f32, tag="vnew_sb")
                    nc.vector.tensor_copy(vnew_sb, pskv[:, HD:2 * HD])

                    # rope on q (scaled by alpha via csq) and k (unscaled)
                    tA = mp.tile([B, OUTW // 2], f32, tag="ropetmp", name="tA")
                    tB = mp.tile([B, OUTW // 2], f32, tag="ropetmp", name="tB")
                    qe, qo = q_sb[:, 0::2], q_sb[:, 1::2]
                    nc.vector.tensor_mul(tA, qe, cq32)
                    nc.vector.tensor_mul(tB, qo, sq32)
                    nc.vector.tensor_tensor(qrot[:, 0::2], tA, tB, SUB)
                    tC = mp.tile([B, OUTW // 2], f32, tag="ropetmp", name="tC")
                    tD = mp.tile([B, OUTW // 2], f32, tag="ropetmp", name="tD")
                    nc.vector.tensor_mul(tC, qe, sq32)
                    nc.vector.tensor_mul(tD, qo, cq32)
                    nc.vector.tensor_add(qrot[:, 1::2], tC, tD)

                    uA = mp.tile([B, HD // 2], f32, tag="kropetmp", name="uA")
                    uB = mp.tile([B, HD // 2], f32, tag="kropetmp", name="uB")
                    ke, ko = k_sb[:, 0::2], k_sb[:, 1::2]
                    nc.vector.tensor_mul(uA, ke, ck32)
                    nc.vector.tensor_mul(uB, ko, sk32)
                    nc.vector.tensor_tensor(krot[:, 0::2], uA, uB, SUB)
                    uC = mp.tile([B, HD // 2], f32, tag="kropetmp", name="uC")
                    uD = mp.tile([B, HD // 2], f32, tag="kropetmp", name="uD")
                    nc.vector.tensor_mul(uC, ke, sk32)
                    nc.vector.tensor_mul(uD, ko, ck32)
                    nc.vector.tensor_add(krot[:, 1::2], uC, uD)

                    # new-token scores: snew[b,h] = sum_d qrot[b,h,d]*krot[b,d]
                    tmp4 = mp.tile([B, HPC, HD], f32, tag="tmp4")
                    nc.vector.tensor_mul(
                        tmp4,
                        qrot[:].rearrange("b (h d) -> b h d", d=HD),
                        krot[:, None, :].to_broadcast([B, HPC, HD]))
                    for h in range(HPC):
                        nc.vector.reduce_sum(snew[:, h:h + 1], tmp4[:, h, :],
                                             axis=X)
                    nc.sync.dma_start(snew_col, snew[:])

                    # transpose q per head -> qxall zero-padded blocks
                    for h in range(HPC):
                        pst = psA.tile([128, B], f32, tag="pstA",
                                       name=f"pstA{h}")
                        nc.tensor.transpose(pst, qrot[:, h * HD:(h + 1) * HD],
                                            iden_sb[0:B, 0:B])
                        nc.vector.tensor_copy(qT_sb[:, h, :], pst)
                    pstv = psA.tile([128, B], f32, tag="pstA")
                    nc.tensor.transpose(pstv, vnew_sb, iden_sb[0:B, 0:B])
                    nc.vector.tensor_copy(vnewT_sb, pstv)

                    for b in range(B):
                        nc.vector.tensor_copy(
                            qxall[:, 128 * b + HPC * b:128 * b
                                  + HPC * (b + 1)],
                            qT_sb[:, :, b])

            # ------- scores + local softmax + interleaved PV (flash halves)
            p16 = pp.tile([128, T], f16, tag="p16")
            maxv = pp.tile([128, 1], f32, tag="maxv")
            negmax = pp.tile([128, 1], f32, tag="negmax")
            sums = pp.tile([128, 1], f32, tag="sums")
            recip = pp.tile([128, 1], f32, tag="recip")
            prow16 = pp.tile([1, 128], f16, tag="prow16")
            rT32 = pp.tile([1, 128], f32, tag="rT32")
            pT = pp.tile([128, PC, 128], f16, tag="pT")
            mh = [pp.tile([128, 1], f32, tag=f"mh{H}", name=f"mh{H}")
                  for H in range(2)]
            v_tiles = {}
            m_c, l_c = [], []

            psat = psP.tile([128, 2, B * HPC], f32, tag="psat")
            psB_cm = tc.tile_pool(name="psB", bufs=1, space="PSUM")
            psB = psB_cm.__enter__()

            def score_pass(H, ps, pqk, after_bg=None):
                """2-chunk-interleaved batch-major scores for one pass.

                Consecutive matmuls share the stationary q block and
                alternate between the pass's two PSUM banks (hides the
                PSUM-RAW / SBUF-access latency). after_bg(bg) lets the
                caller interleave PV work between batch groups.
                """
                for bg in range(4):
                    t_i = 8 * H + 4 * ps + bg
                    tkb = kt_tiles.pop(t_i)
                    for j in range(8):
                        b = bg * 8 + j
                        for cl in range(2):
                            nc.tensor.matmul(
                                pqk[:, cl, :],
                                qxall[:, 128 * b:128 * (b + 1)],
                                tkb[:, j, cl, :],
                                start=(b == 0), stop=(b == B - 1),
                                skip_group_check=True)
                    nt = t_i + KBUFS
                    if nt < 16:
                        t2 = ktp.tile([128, 8, 2, 512], f16, tag="ktb",
                                      name=f"ktb{nt}")
                        nc.sync.dma_start(t2, ktv[nt])
                        kt_tiles[nt] = t2
                    if after_bg is not None:
                        after_bg(bg)
                for cl in range(2):
                    c = 4 * H + 2 * ps + cl
                    if c == TC - 1:
                        # zero the stale col-4095 score: its exp contributes
                        # only e^-m_c to the row sum (negligible)
                        nc.vector.tensor_copy(pqk[:, cl, 511:512], zero1)
                    mc = mp.tile([128, 1], f32, tag="mxc", name=f"mx{c}",
                                 bufs=TC)
                    nc.vector.reduce_max(mc, pqk[:, cl, :], axis=X)
                    ngc = mp.tile([128, 1], f32, tag="ngc", name=f"ng{c}",
                                  bufs=TC)
                    nc.vector.tensor_scalar_mul(ngc, mc, -1.0)
                    lc = mp.tile([128, 1], f32, tag="sumc", name=f"sum{c}",
                                 bufs=TC)
                    nc.scalar.activation(p16[:, c * 512:(c + 1) * 512],
                                         pqk[:, cl, :], EXP, bias=ngc,
                                         scale=1.0, accum_out=lc)
                    m_c.append(mc)
                    l_c.append(lc)

            def rescale_transpose_half(H):
                """gamma_c rescale to the half max + pT transposes."""
                nc.vector.tensor_tensor(mh[H], m_c[4 * H], m_c[4 * H + 1],
                                        MAX)
                nc.vector.tensor_tensor(mh[H], mh[H], m_c[4 * H + 2], MAX)
                nc.vector.tensor_tensor(mh[H], mh[H], m_c[4 * H + 3], MAX)
                for c in range(4 * H, 4 * H + 4):
                    gs = mp.tile([128, 1], f32, tag="gsc", name=f"gs{c}",
                                 bufs=TC)
                    nc.vector.tensor_tensor(gs, m_c[c], mh[H], SUB)
                    gc = mp.tile([128, 1], f32, tag="gc", name=f"g{c}",
                                 bufs=TC)
                    nc.scalar.activation(gc, gs, EXP)
                    g_c.append(gc)
                    nc.vector.tensor_scalar_mul(
                        p16[:, c * 512:(c + 1) * 512],
                        p16[:, c * 512:(c + 1) * 512], gc)
                for c2 in range(PCH * H, PCH * (H + 1)):
                    pstx = psT.tile([128, 128], f16, tag="pstx",
                                    name=f"pstx{c2}")
                    nc.tensor.transpose(pstx, p16[:, c2 * 128:(c2 + 1) * 128],
                                        iden16_sb)
                    nc.vector.tensor_copy(pT[:, c2, :], pstx)

            def pv_batches(H, b0, b1):
                """PV matmuls for batches [b0,b1) of half H + V recycling.

                V tiles stream in consumption order: flat index 8*H + bq.
                """
                for b in range(b0, b1):
                    bq, lane = b // 4, b % 4
                    t_i = 8 * H + bq
                    vb = v_tiles[t_i]
                    for cl in range(PCH):
                        nc.tensor.matmul(
                            psat[:, H, HPC * b:HPC * (b + 1)],
                            vb[:, lane, cl, :],
                            pT[:, PCH * H + cl, HPC * b:HPC * (b + 1)],
                            start=(cl == 0), stop=(cl == PCH - 1),
                            skip_group_check=True)
                    if lane == 3:
                        del v_tiles[t_i]
                        nt = t_i + VBUFS
                        if nt < 16:
                            nH, nbq = divmod(nt, 8)
                            v2t = vqp.tile([128, 4, PCH, HD], vdt, tag="vq",
                                           name=f"vq{nt}")
                            nc.gpsimd.dma_start(v2t, vcv[nbq, nH])
                            v_tiles[nt] = v2t

            g_c = []
            # half 0 scores: two 2-chunk passes
            for ps in range(2):
                pqk = psB.tile([128, 2, 512], f32, tag="pqk", bufs=2,
                               name=f"pqk0{ps}")
                score_pass(0, ps, pqk)
            # V tile order: flat index 8*H+bq (all half-0 quads first);
            # prefetch first VBUFS (gpsimd queue) -- streams beside K-half-1
            for t in range(VBUFS):
                vb = vqp.tile([128, 4, PCH, HD], vdt, tag="vq",
                              name=f"vq{t}")
                nc.gpsimd.dma_start(vb, vcv[t, 0])
                v_tiles[t] = vb
            rescale_transpose_half(0)

            # half 1 scores interleaved with PV half 0 (4 batches per
            # batch-group round)
            for ps in range(2):
                pqk = psB.tile([128, 2, 512], f32, tag="pqk", bufs=2,
                               name=f"pqk1{ps}")
                score_pass(1, ps, pqk,
                           after_bg=lambda bg, _ps=ps: pv_batches(
                               0, _ps * 16 + bg * 4, _ps * 16 + bg * 4 + 4))
            psB_cm.__exit__(None, None, None)
            ktp_cm.__exit__(None, None, None)

            # wo preload in the freed K region
            wopool_cm = tc.tile_pool(name="wopool", bufs=1)
            wopool = wopool_cm.__enter__()
            wo_sb = wopool.tile([128, HPC, DIM], f16, tag="wo_sb")
            wov = wo[:].rearrange("p (h o) -> p h o", o=DIM)
            for h in range(HPC):
                nc.gpsimd.dma_start(wo_sb[:, h, :], wov[:, h, :])

            rescale_transpose_half(1)
            # PV half 1
            pv_batches(1, 0, B)

            # ------- merge halves + correction + output projection
            nc.vector.tensor_tensor(maxv, mh[0], mh[1], MAX)
            nc.vector.tensor_tensor(maxv, maxv, snew_col, MAX)
            nc.vector.tensor_scalar_mul(negmax, maxv, -1.0)
            # alpha_H = e^{m_H - m}; row sums = sum_H alpha_H sum_c l_c g_c
            aH = []
            for H in range(2):
                as_ = mp.tile([128, 1], f32, tag="asH", name=f"as{H}",
                              bufs=2)
                nc.vector.tensor_tensor(as_, mh[H], maxv, SUB)
                a_ = mp.tile([128, 1], f32, tag="aH", name=f"a{H}", bufs=2)
                nc.scalar.activation(a_, as_, EXP)
                aH.append(a_)
            lg = mp.tile([128, 1], f32, tag="lg")
            lh = mp.tile([128, 1], f32, tag="lh")
            first = True
            for H in range(2):
                for c in range(4 * H, 4 * H + 4):
                    nc.vector.tensor_mul(lg, l_c[c], g_c[c])
                    if c % 4 == 0:
                        nc.vector.tensor_copy(lh, lg)
                    else:
                        nc.vector.tensor_add(lh, lh, lg)
                nc.vector.tensor_mul(lh, lh, aH[H])
                if first:
                    nc.vector.tensor_copy(sums, lh)
                    first = False
                else:
                    nc.vector.tensor_add(sums, sums, lh)
            # new-token exp -> p~ col (global max)
            nc.scalar.activation(p16[:, T - 1:T], snew_col, EXP, bias=negmax,
                                 scale=1.0)
            pcol32 = mp.tile([128, 1], f32, tag="pcol32")
            nc.vector.tensor_copy(pcol32, p16[:, T - 1:T])
            nc.vector.tensor_add(sums, sums, pcol32)
            nc.vector.reciprocal(recip, sums)

            with tc.tile_pool(name="psC", bufs=2, space="PSUM") as psC:
                psr = psC.tile([1, 128], f16, tag="psrow", bufs=1,
                               name="psr")
                nc.tensor.transpose(psr, p16[:, T - 1:T], iden16_sb)
                nc.vector.tensor_copy(prow16, psr)

                pstr = psC.tile([1, 128], f32, tag="psrow", bufs=1,
                                name="pstr")
                nc.tensor.transpose(pstr, recip, iden_sb)
                nc.vector.tensor_copy(rT32, pstr)

                # rank-1 column broadcasts: alpha_0, alpha_1, p~row, recip
                a0T = mp.tile([1, 128], f32, tag="a0T")
                psa0 = psC.tile([1, 128], f32, tag="psrow", bufs=1,
                                name="psa0")
                nc.tensor.transpose(psa0, aH[0], iden_sb)
                nc.vector.tensor_copy(a0T, psa0)
                a1T = mp.tile([1, 128], f32, tag="a1T")
                psa1 = psC.tile([1, 128], f32, tag="psrow", bufs=1,
                                name="psa1")
                nc.tensor.transpose(psa1, aH[1], iden_sb)
                nc.vector.tensor_copy(a1T, psa1)

                # serialized rank-1 broadcasts (one PSUM bank each, consumer
                # immediately after producer so the bank can rotate)
                psbc1 = psC.tile([128, 128], f32, tag="psbc", bufs=1,
                                 name="psbc1")
                nc.tensor.matmul(psbc1, ones16_sb, prow16)
                corrT = mp.tile([128, B, HPC], f32, tag="corrT")
                nc.vector.tensor_mul(
                    corrT,
                    vnewT_sb[:, :, None].to_broadcast([128, B, HPC]),
                    psbc1[:].rearrange("d (b h) -> d b h", h=HPC))
                psba0 = psC.tile([128, 128], f32, tag="psba", bufs=1,
                                 name="psba0")
                nc.tensor.matmul(psba0, ones32_sb, a0T)
                at_f = mp.tile([128, B * HPC], f32, tag="at_f")
                nc.vector.tensor_copy(at_f, psat[:, 0, :])
                nc.vector.tensor_mul(at_f, at_f, psba0)
                psba1 = psC.tile([128, 128], f32, tag="psba", bufs=1,
                                 name="psba1")
                nc.tensor.matmul(psba1, ones32_sb, a1T)
                at_g = mp.tile([128, B * HPC], f32, tag="at_g")
                nc.vector.tensor_copy(at_g, psat[:, 1, :])
                nc.vector.tensor_mul(at_g, at_g, psba1)
                nc.vector.tensor_add(at_f, at_f, at_g)
                nc.vector.tensor_add(
                    at_f, at_f, corrT[:].rearrange("d b h -> d (b h)"))
                psbc2 = psC.tile([128, 128], f32, tag="psbc", bufs=1,
                                 name="psbc2")
                nc.tensor.matmul(psbc2, ones32_sb, rT32)
                attnT = pp.tile([128, B * HPC], f16, tag="attnT")
                nc.vector.tensor_mul(attnT, at_f, psbc2)

                # out projection
                for ncc in range(8):
                    pso = psC.tile([B, 512], f32, tag="pso", name=f"pso{ncc}")
                    for h in range(HPC):
                        nc.tensor.matmul(
                            pso, attnT[:, h::HPC],
                            wo_sb[:, h, ncc * 512:(ncc + 1) * 512],
                            start=(h == 0), stop=(h == HPC - 1))
                    osb = outpp.tile([B, 512], f32, tag="osb",
                                     name=f"osb{ncc}")
                    nc.vector.tensor_copy(osb, pso)
                    nc.sync.dma_start(outp[:, ncc * 512:(ncc + 1) * 512], osb)

            wopool_cm.__exit__(None, None, None)
            psT_cm.__exit__(None, None, None)
            psP_cm.__exit__(None, None, None)

    nc.compile()
    return nc


def make_in_maps(inputs):
    x = np.asarray(inputs["x"], np.float32).reshape(B, DIM)
    cache_k = np.asarray(inputs["cache_k"], np.float32)
    cache_v = np.asarray(inputs["cache_v"], np.float32)
    wq = np.asarray(inputs["wq"], np.float32)
    wk = np.asarray(inputs["wk"], np.float32)
    wv = np.asarray(inputs["wv"], np.float32)
    wo = np.asarray(inputs["wo"], np.float32)
    cos = np.asarray(inputs["freqs_cos"], np.float32).reshape(-1)
    sin = np.asarray(inputs["freqs_sin"], np.float32).reshape(-1)

    f16 = np.float16
    vdt = ml_dtypes.float8_e3m4 if V_FP8 else f16
    xT = np.ascontiguousarray(
        x.T.reshape(DC, 128, B).transpose(1, 0, 2)
        .reshape(128, DC * B)).astype(f16)                     # [128, DC*B]
    csq = np.ascontiguousarray(
        np.stack([np.tile(cos, HPC), np.tile(sin, HPC)]) * ALPHA)
    csk = np.ascontiguousarray(np.stack([cos, sin]))
    ones16v = np.ones((1, 128), f16)
    ones32v = np.ones((1, 128), np.float32)
    idenv = np.eye(128, dtype=np.float32)
    iden16v = np.eye(128, dtype=f16)

    v8 = cache_v.astype(vdt)                                   # quantize once

    in_maps = []
    for g in range(NCORES):
        wq_g = wq[:, g * OUTW:(g + 1) * OUTW]
        wq_pre = np.ascontiguousarray(
            wq_g.reshape(DC, 128, OUTW).transpose(1, 0, 2)
            .reshape(128, DC * OUTW)).astype(f16)
        wk_r = wk[:, g * HD:(g + 1) * HD].reshape(DC, 128, HD)
        wv_r = wv[:, g * HD:(g + 1) * HD].reshape(DC, 128, HD)
        wkv_pre = np.ascontiguousarray(
            np.stack([wk_r, wv_r], axis=2).transpose(1, 0, 2, 3)
            .reshape(128, DC * 2 * HD)).astype(f16)
        wo_g = wo[g * OUTW:(g + 1) * OUTW, :]
        wo_pre = np.ascontiguousarray(
            wo_g.reshape(HPC, 128, DIM).transpose(1, 0, 2)
            .reshape(128, HPC * DIM)).astype(f16)
        # K tiles [(H,pass,bg), d, j, c, n]: b = bg*8+j,
        # t = H*2048 + (2*pass+c)*512 + n
        kt_g = np.ascontiguousarray(
            cache_k[:, :, g, :].reshape(4, 8, 2, 2, 2, 512, HD)
            .transpose(2, 3, 0, 6, 1, 4, 5)
            .reshape(16, 128, 8 * 2 * 512)).astype(f16)
        # V: quad-batch half tiles [bq, H, r, lane, chunk, d]
        v_g = np.ascontiguousarray(
            v8[:, :, g, :].reshape(B // 4, 4, 2, PCH, 128, HD)
            .transpose(0, 2, 4, 1, 3, 5)
            .reshape(B // 4, 2, 128, 4 * PCH * HD))
        in_maps.append({
            "xT": xT,
            "wq": wq_pre,
            "wkv": wkv_pre,
            "wo": wo_pre,
            "kt": kt_g,
            "vc": v_g,
            "csq": csq,
            "csk": csk,
            "ones16": ones16v,
            "ones32": ones32v,
            "iden": idenv,
            "iden16": iden16v,
        })
    return in_maps


_NC_CACHE = []


def run(inputs, trace=False, **kwargs):
    from concourse.bass_utils import run_bass_kernel_spmd
    if not _NC_CACHE:
        _NC_CACHE.append(build_nc())
    nc = _NC_CACHE[0]
    in_maps = make_in_maps(inputs)
    res = run_bass_kernel_spmd(nc, in_maps, core_ids=list(range(NCORES)),
                               trace=trace, **kwargs)
    partials = np.stack([r["outp"] for r in res.results])      # [8, B, DIM]
    out = partials.sum(axis=0, dtype=np.float64).astype(np.float32)
    return out.reshape(B, 1, DIM), res


def kernel(**inputs):
    out, _ = run(inputs)
    return out
